# revision 36
# baseline (speedup 1.0000x reference)
"""Trainium2 Bass kernel for a dense transformer block (B=4, T=2048, C=384, H=6).

Sharding: 8 cores; core c handles batch c//2 and heads [3*(c%2), 3*(c%2)+3).
Each core computes LN1 + QKV + causal attention + its partial of the output
projection for its 3 heads over the full sequence; a pairwise ReduceScatter
combines projection partials and splits tokens; each core then runs
LN2 + FFN + residuals for its half of the tokens and outputs [T/2, C].

v2 design notes (vs the v1 baseline):
- matmul operand tiles in bf16 (PSUM accumulation stays f32); residual
  stream and bounce buffers stay f32.
- LN via bn_stats/bn_aggr; LN gains folded into adjacent weights on host.
- Q/K/V biases eliminated on-chip: V-bias folds into the residual row on
  host; K-bias cancels in softmax (constant per query); Q-bias is applied
  via a K=1 ones-row matmul accumulated into the Q projection.
- V computed directly token-partitioned ([s, d]) so no V transpose pass.
- Scores computed per (head, query-block of 256) into [128, 512] PSUM
  pair-tiles (two 128-wide key blocks side by side) -> one exp per pair.
- o-matmul flipped: pt is the stationary operand, V (66 cols incl ones
  column) streams -> output o^T [q, 66] with the softmax denominator in
  column 64, so normalization is a per-partition scale at eviction; a
  [128,64] transpose-back rebuilds the [d, q] layout for the projection.
- Eviction copies distributed across Pool/DVE/Act engines.
"""
import math

import numpy as np

import concourse.bass as bass
import concourse.mybir as mybir
import concourse.tile as tile
from concourse import bacc, bass_utils
from concourse.masks import make_identity

F32 = mybir.dt.float32
BF16 = mybir.dt.bfloat16
F32R = mybir.dt.float32r

B, T, C, H = 4, 2048, 384, 6
HS = C // H  # 64
NHC = 3      # heads per core
QW = 256     # query-block width (2 PSUM o-tiles of 128)
SW = 128     # key-block width
EPS = 1e-5
RS_CHUNKS = 2  # reduce-scatter split; core owns chunk_base + half*(T/2/RS_CHUNKS)


def core_token_slices(half, T=T):
    """Global token ranges owned by a core after the chunked ReduceScatter."""
    th_c = T // 2 // RS_CHUNKS  # tokens per chunk per core
    return [(ch * (T // RS_CHUNKS) + half * th_c, th_c)
            for ch in range(RS_CHUNKS)]


# --------------------------------------------------------------------------
# Bass program builder (uniform SPMD program; per-core data differs)
# --------------------------------------------------------------------------

def build_block(T=T, num_devices=8, replica_groups=None, mm="bf16",
                skip_rs=False, phases="all", iters=1):
    if replica_groups is None:
        replica_groups = [[2 * i, 2 * i + 1] for i in range(num_devices // 2)]
    MMDT = BF16
    TH = T // 2
    NQ = T // QW     # 8 query blocks
    NTB = T // 128   # 16 token blocks
    C4 = 4 * C       # 1536
    NCC = C // 128   # 3 contraction chunks
    NFC = C4 // 128  # 12 ffn chunks
    AF = mybir.ActivationFunctionType
    ALU = mybir.AluOpType

    nc = bacc.Bacc("TRN2", target_bir_lowering=False, debug=False,
                   enable_asserts=False, num_devices=num_devices)

    def din(name, shape):
        return nc.dram_tensor(name, shape, F32, kind="ExternalInput").ap()

    x = din("x", (T, C))
    xpb = din("xpb", (TH, C))
    wq01 = din("wq01", (C, 128))
    wk01 = din("wk01", (C, 128))
    wqk2 = din("wqk2", (C, 128))
    wvall = din("wvall", (C, 192))
    bqcol = din("bqcol", (256,))
    wp01 = din("wp01", (128, C))
    wp2 = din("wp2", (64, C))
    w1 = din("w1", (C, C4))
    b1 = din("b1", (C4,))
    w2 = din("w2", (C4, C))
    b2 = din("b2", (C,))
    maskAB = din("maskAB", (SW, 2 * QW))
    y = nc.dram_tensor("y", (TH, C), F32, kind="ExternalOutput").ap()

    with tile.TileContext(nc) as tc:
        with tc.tile_pool(name="const", bufs=1) as cp, \
             tc.tile_pool(name="stage", bufs=1) as sp, \
             tc.tile_pool(name="psum", bufs=1, space="PSUM") as pp, \
             tc.tile_pool(name="dram", bufs=2, space="DRAM") as dp, \
             tc.tile_pool(name="actv", bufs=1) as avp, \
             tc.tile_pool(name="worka", bufs=4) as wpa, \
             tc.tile_pool(name="att", bufs=4) as ap_, \
             tc.tile_pool(name="worke", bufs=2) as wpe:

            # ---------------- constants / weights ----------------
            identB = cp.tile([128, 128], BF16)
            make_identity(nc, identB)
            epsB = cp.tile([128, 1], F32)
            nc.gpsimd.memset(epsB, EPS)
            # (1, 0) pairs at cols 64,65 of each head's 66-wide V block
            vpat3 = cp.tile([128, NTB * NHC * 2], BF16)
            nc.gpsimd.memset(vpat3, 0.0)
            nc.gpsimd.memset(
                vpat3.rearrange("p (s h w) -> p s h w", h=NHC, w=2)[:, :, :, 0:1],
                1.0)

            def load_const(dram_ap, shape, name, dt=MMDT, rearr=None):
                """Stage f32 DRAM into an on-chip tile, converting to the
                matmul dtype (chunked along the middle dim for 3-D tiles)."""
                t = cp.tile(list(shape), dt, name=name)
                src = dram_ap if rearr is None else rearr
                if dt == F32:
                    nc.sync.dma_start(t[:], src)
                elif len(shape) == 3:
                    for j in range(shape[1]):
                        s = sp.tile([shape[0], shape[2]], F32,
                                    name=f"{name}_s{j}", tag="wstage")
                        nc.sync.dma_start(s[:], src[:, j, :])
                        nc.vector.tensor_copy(t[:, j, :], s[:])
                else:
                    s = sp.tile(list(shape), F32, name=name + "_s", tag="wstage2")
                    nc.sync.dma_start(s[:], src)
                    nc.vector.tensor_copy(t[:], s[:])
                return t

            def load_wqkv(dram_ap, width, name):
                return load_const(
                    dram_ap, (128, NCC, width), name, dt=MMDT,
                    rearr=dram_ap.rearrange("(c p) w -> p c w", p=128))

            wq01m = load_wqkv(wq01, 128, "wq01m")
            wk01m = load_wqkv(wk01, 128, "wk01m")
            wqk2m = load_wqkv(wqk2, 128, "wqk2m")
            wvallm = load_wqkv(wvall, 192, "wvallm")
            wp01m = load_const(wp01, (128, C), "wp01m")
            wp2m = load_const(wp2, (64, C), "wp2m")
            w1m = load_const(w1, (128, NCC, C4), "w1m",
                             rearr=w1.rearrange("(c p) f -> p c f", p=128))
            w2m = load_const(w2, (128, NFC, C), "w2m",
                             rearr=w2.rearrange("(f p) c -> p f c", p=128))
            maskm = load_const(maskAB, (SW, 2 * QW), "maskm")
            bq01col = cp.tile([128, 1], F32)
            nc.sync.dma_start(bq01col[:], bqcol[0:128, None])
            bq2col = cp.tile([64, 1], F32)
            nc.sync.dma_start(bq2col[:], bqcol[128:192, None])

            b1s = cp.tile([128, NFC], F32)
            nc.sync.dma_start(b1s[:], b1.rearrange("(f p) -> p f", p=128))
            b2row = cp.tile([1, C], F32)
            nc.sync.dma_start(b2row[:], b2[None, :])
            b2bc = cp.tile([128, C], F32)
            nc.gpsimd.partition_broadcast(b2bc[:], b2row[:])

            def get_bounce():
                bin0 = dp.tile([T // 2, C], F32, tag="bin0", name="bounce_in0")
                bin1 = dp.tile([T // 2, C], F32, tag="bin1", name="bounce_in1")
                bout0 = dp.tile([TH // 2, C], F32, tag="bout0",
                                name="bounce_out0")
                bout1 = dp.tile([TH // 2, C], F32, tag="bout1",
                                name="bounce_out1")
                return (bin0, bin1), (bout0, bout1)

            def dump_y(tile_ap):
                for _tb in range(TH // 128):
                    nc.gpsimd.dma_start(y[_tb * 128:(_tb + 1) * 128, :],
                                        tile_ap)

            # ---------------- helpers ----------------
            def ln_stats(wp, xt, tag):
                """token-major [128, C] f32 -> (rstd, nmr) [128,1] scalars."""
                s6 = wp.tile([128, 6], F32, tag=tag + "_s6")
                nc.vector.bn_stats(s6[:], xt)
                mv = wp.tile([128, 2], F32, tag=tag + "_mv")
                nc.vector.bn_aggr(mv[:], s6[:])
                std = wp.tile([128, 1], F32, tag=tag + "_std")
                nc.scalar.activation(std[:], mv[:, 1:2], AF.Sqrt, bias=epsB[:])
                rstd = wp.tile([128, 1], F32, tag=tag + "_rstd")
                nc.vector.reciprocal(rstd[:], std[:])
                nmr = wp.tile([128, 1], F32, tag=tag + "_nmr")
                nc.vector.tensor_scalar(nmr[:], mv[:, 0:1], rstd[:], -1.0,
                                        op0=ALU.mult, op1=ALU.mult)
                return rstd, nmr

            if phases == "dma":
                xdump = sp.tile([128, C], F32, name="xdump", tag="wstage")
                nc.sync.dma_start(xdump[:], x[0:128, :])
                xp2 = sp.tile([128, C], F32, name="xp2", tag="wstage")
                nc.sync.dma_start(xp2[:], xpb[0:128, :])
                nc.vector.tensor_add(xdump[:], xdump[:], b2bc[:])
                nc.vector.tensor_add(xdump[:], xdump[:], xp2[:])
                dump_y(xdump[:])

            def emit_AB_alloc():
                t = {}
                t["xnT"] = avp.tile([128, NCC, T], MMDT, name="xnT")
                t["qt01"] = avp.tile([128, T], MMDT, name="qt01", bufs=2)
                t["kt01"] = avp.tile([128, T], MMDT, name="kt01", bufs=2)
                t["qt2"] = avp.tile([64, T], MMDT, name="qt2", bufs=2)
                t["kt2"] = avp.tile([64, T], MMDT, name="kt2", bufs=2)
                t["vaugC"] = avp.tile([128, NTB, NHC * 66], MMDT,
                                      name="vaugC", bufs=2)
                return t

            def emit_A_tb(t, tb):
                xnT = t["xnT"]
                xt = wpa.tile([128, C], F32, tag="xt")
                nc.gpsimd.dma_start(xt[:], x[tb * 128:(tb + 1) * 128, :])
                rstd, nmr = ln_stats(wpa, xt[:], "ln1")
                n = wpa.tile([128, C], MMDT, tag="ln_n")
                nc.gpsimd.tensor_scalar(n[:], xt[:], rstd[:], nmr[:],
                                        op0=ALU.mult, op1=ALU.add)
                ptb = pp.tile([128, NCC, 128], MMDT, tag="atr", bufs=1)
                for cc in range(NCC):
                    nc.tensor.transpose(ptb[:, cc, :],
                                        n[:, cc * 128:(cc + 1) * 128],
                                        identB[:])
                nc.vector.tensor_copy(
                    xnT[:, :, tb * 128:tb * 128 + 128], ptb[:])

            _QKV = ((0, 0, 128, "qt01", 1, "dve"),
                    (1, 0, 128, "kt01", None, "dve"),
                    (2, 0, 64, "qt2", 2, "pool"),
                    (2, 64, 64, "kt2", None, "pool"))
            _WMS = None

            def emit_B_qkv(t, gi, tch):
                wms = (wq01m, wk01m, wqk2m)
                wi, w0, gw, dname, bci, eng = _QKV[gi]
                wm = wms[wi]
                bcol = (None, bq01col, bq2col)[bci] if bci else None
                dst = t[dname]
                xnT = t["xnT"]
                ps = pp.tile([gw, 512], F32, tag="mm", bufs=2)
                for cc in range(NCC):
                    nc.tensor.matmul(
                        ps[:], wm[:, cc, w0:w0 + gw],
                        xnT[:, cc, tch * 512:(tch + 1) * 512],
                        start=(cc == 0), stop=(cc == NCC - 1))
                dsl = dst[:, tch * 512:(tch + 1) * 512]
                if eng == "dve":
                    if bcol is not None:
                        nc.vector.tensor_scalar(dsl, ps[:], bcol[:], None,
                                                op0=ALU.add)
                    else:
                        nc.vector.tensor_copy(dsl, ps[:])
                else:
                    if bcol is not None:
                        nc.vector.tensor_scalar(dsl, ps[:], bcol[:], None,
                                                op0=ALU.add)
                    else:
                        nc.vector.tensor_copy(dsl, ps[:])

            def emit_B_v(t, sb):
                xnT, vaugC = t["xnT"], t["vaugC"]
                vps = pp.tile([128, 192], F32, tag="mm", bufs=2)
                for cc in range(NCC):
                    nc.tensor.matmul(
                        vps[:], xnT[:, cc, sb * 128:sb * 128 + 128],
                        wvallm[:, cc, :],
                        start=(cc == 0), stop=(cc == NCC - 1))
                nc.vector.tensor_copy(
                    vaugC[:, sb, :].rearrange(
                        "p (h w) -> p h w", w=66)[:, :, 0:64],
                    vps[:].rearrange("p (h w) -> p h w", w=64))

            def emit_B_vpat(t):
                vaugC = t["vaugC"]
                nc.vector.tensor_copy(
                    vaugC.rearrange("p s (h w) -> p s h w", w=66)[:, :, :,
                                                                  64:66],
                    vpat3.rearrange("p (s h w) -> p s h w", h=NHC, w=2))

            def b_units(t):
                for gi in range(4):
                    for tch in range(T // 512):
                        yield lambda gi=gi, tch=tch: emit_B_qkv(t, gi, tch)
                for sb in range(NTB):
                    yield lambda sb=sb: emit_B_v(t, sb)
                yield lambda: emit_B_vpat(t)

            def emit_C_qi(t, bounce_in, qi):
                head_qt = [(t["qt01"], 0), (t["qt01"], 64), (t["qt2"], 0)]
                head_kt = [(t["kt01"], 0), (t["kt01"], 64), (t["kt2"], 0)]
                vaugC = t["vaugC"]
                oT01, oT2 = t["oT01"], t["oT2"]
                opsall = pp.tile([128, 2 * NHC, 66], F32, tag="o", bufs=1)
                nc.vector.memset(opsall[:], 0.0)
                o_ps = [[opsall[:, 2 * h + qb, :] for qb in range(2)]
                        for h in range(NHC)]
                for p in range(qi + 1):
                    for h in range(NHC):
                        qtile, qoff = head_qt[h]
                        ktile, koff = head_kt[h]
                        sc = pp.tile([128, 512], F32, tag="sc", bufs=2)
                        for half in range(2):
                            s0 = (2 * p + half) * SW
                            nc.tensor.matmul(
                                sc[:, half * 256:(half + 1) * 256],
                                ktile[koff:koff + 64, s0:s0 + SW],
                                qtile[qoff:qoff + 64,
                                      qi * QW:(qi + 1) * QW],
                                start=True, stop=True)
                        pt = ap_.tile([128, 512], MMDT, tag="pt", bufs=8)
                        nc.scalar.activation(pt[:], sc[:], AF.Exp,
                                             scale=float(HS) ** -0.5)
                        if p == qi:
                            nc.vector.tensor_mul(pt[:], pt[:], maskm[:])
                        for half in range(2):
                            vsl = vaugC[:, 2 * p + half, h * 66:h * 66 + 66]
                            for qb in range(2):
                                nc.tensor.matmul(
                                    o_ps[h][qb],
                                    pt[:, half * 256 + qb * 128:
                                       half * 256 + qb * 128 + 128],
                                    vsl,
                                    start=False,
                                    stop=(p == qi and half == 1 and h ==
                                          NHC - 1 and qb == 1))
                for h in range(NHC):
                    tbp = pp.tile([64, 256], MMDT, tag="tb", bufs=1)
                    for qb in range(2):
                        ops = o_ps[h][qb]
                        rd = ap_.tile([128, 1], F32, tag="rd", bufs=6)
                        nc.vector.reciprocal(rd[:], ops[:, 64:65])
                        oq = ap_.tile([128, 64], MMDT, tag="oq", bufs=6)
                        nc.vector.tensor_scalar(oq[:], ops[:, 0:64], rd[:],
                                                None, op0=ALU.mult)
                        nc.tensor.transpose(
                            tbp[:, qb * 128:(qb + 1) * 128], oq[:], identB[:])
                    if h < 2:
                        dst = oT01[h * 64:(h + 1) * 64, qi * QW:(qi + 1) * QW]
                    else:
                        dst = oT2[:, qi * QW:(qi + 1) * QW]
                    nc.vector.tensor_copy(dst, tbp[:])
                for ts in range(2):
                    t0 = qi * QW + ts * 128
                    pj = pp.tile([128, C], F32, tag="mm", bufs=2)
                    nc.tensor.matmul(pj[:], oT01[:, t0:t0 + 128], wp01m[:],
                                     start=True, stop=False)
                    nc.tensor.matmul(pj[:], oT2[:, t0:t0 + 128], wp2m[:],
                                     start=False, stop=True)
                    pstage = ap_.tile([128, C], F32, tag="pstage", bufs=4)
                    nc.vector.tensor_copy(pstage[:], pj[:])
                    bch, boff = divmod(t0, T // 2)
                    nc.sync.dma_start(
                        bounce_in[bch][boff:boff + 128, :], pstage[:])

            def emit_RS(bounce_in, bounce_out, ch):
                if skip_rs:
                    nc.sync.dma_start(bounce_out[ch][:, :],
                                      bounce_in[ch][0:TH // 2, :])
                else:
                    nc.gpsimd.collective_compute(
                        "ReduceScatter", mybir.AluOpType.add,
                        replica_groups=replica_groups,
                        ins=[bounce_in[ch].opt()],
                        outs=[bounce_out[ch].opt()])

            def emit_E_t2(bounce_out, t2):
                r0 = t2 * 512
                rsin = wpe.tile([128, 4, C], F32, tag="rsin", bufs=2)
                xpbt = wpe.tile([128, 4, C], F32, tag="xpbt", bufs=2)
                xmid = rsin
                nc.scalar.dma_start(
                    rsin[:],
                    bounce_out[t2][:, :].rearrange("(s p) c -> p s c", p=128))
                nc.scalar.dma_start(
                    xpbt[:],
                    xpb[r0:r0 + 512, :].rearrange("(s p) c -> p s c", p=128))
                nc.gpsimd.tensor_add(xmid[:], rsin[:], xpbt[:])
                n2T = wpe.tile([128, NCC, 512], MMDT, tag="n2T", bufs=2)
                for ts in range(4):
                    xsl = xmid[:, ts, :]
                    rstd, nmr = ln_stats(wpe, xsl, "ln2")
                    n2 = wpe.tile([128, C], MMDT, tag="n2")
                    nc.gpsimd.tensor_scalar(n2[:], xsl, rstd[:], nmr[:],
                                            op0=ALU.mult, op1=ALU.add)
                    ptb = pp.tile([128, NCC, 128], MMDT, tag="etr", bufs=1)
                    for cc in range(NCC):
                        nc.tensor.transpose(
                            ptb[:, cc, :], n2[:, cc * 128:(cc + 1) * 128],
                            identB[:])
                    nc.vector.tensor_copy(
                        n2T[:, :, ts * 128:ts * 128 + 128], ptb[:])
                h1T = wpe.tile([128, NFC, 512], MMDT, tag="h1T", bufs=2)
                for fc in range(NFC):
                    fps = pp.tile([128, 512], F32, tag="mm", bufs=2)
                    for cc in range(NCC):
                        nc.tensor.matmul(
                            fps[:], w1m[:, cc, fc * 128:(fc + 1) * 128],
                            n2T[:, cc, :],
                            start=(cc == 0), stop=(cc == NCC - 1))
                    if t2 == 0:
                        nc.vector.tensor_scalar(h1T[:, fc, :], fps[:],
                                                b1s[:, fc:fc + 1], 0.0,
                                                op0=ALU.add, op1=ALU.max)
                    else:
                        nc.scalar.activation(h1T[:, fc, :], fps[:], AF.Relu,
                                             bias=b1s[:, fc:fc + 1])
                for ts in range(4):
                    yps = pp.tile([128, C], F32, tag="mm", bufs=2)
                    for fc in range(NFC):
                        nc.tensor.matmul(
                            yps[:], h1T[:, fc, ts * 128:ts * 128 + 128],
                            w2m[:, fc, :],
                            start=(fc == 0), stop=(fc == NFC - 1))
                    ot = wpe.tile([128, C], F32, tag="ot", bufs=3)
                    nc.vector.scalar_tensor_tensor(
                        ot[:], yps[:], 1.0, xmid[:, ts, :],
                        op0=ALU.mult, op1=ALU.add)
                    nc.gpsimd.tensor_add(ot[:], ot[:], b2bc[:])
                    nc.sync.dma_start(
                        y[r0 + ts * 128: r0 + ts * 128 + 128, :], ot[:])

            # ---- software-pipelined emission over iterations ----
            n_iters = iters if phases != "dma" else 0
            tiles_cur = None
            for _it in range(n_iters):
                if tiles_cur is None:
                    tiles_cur = emit_AB_alloc()
                    for tb in range(NTB):
                        emit_A_tb(tiles_cur, tb)
                    for u in b_units(tiles_cur):
                        u()
                tiles_cur["oT01"] = avp.tile([128, T], MMDT, name="oT01")
                tiles_cur["oT2"] = avp.tile([64, T], MMDT, name="oT2")
                bounce_in, bounce_out = get_bounce()
                if phases == "ab":
                    dump_y(tiles_cur["qt01"][:, 0:C])
                    continue
                # next iteration's A/B emitted interleaved into this C
                tiles_next = emit_AB_alloc() if _it + 1 < n_iters else None
                nxt = []
                if tiles_next is not None:
                    nxt = [lambda tb=tb: emit_A_tb(tiles_next, tb)
                           for tb in range(NTB)] + list(b_units(tiles_next))
                # ~distribute across qi 1..7 (after qi0 C emission)
                ni = 0
                for qi in range(NQ):
                    emit_C_qi(tiles_cur, bounce_in, qi)
                    if qi >= 1:
                        take = (len(nxt) * min(2 * qi, NQ - 1)) // (NQ - 1) - ni
                        for _ in range(take):
                            nxt[ni]()
                            ni += 1
                    if qi == 3:
                        emit_RS(bounce_in, bounce_out, 0)
                        if phases == "all":
                            emit_E_t2(bounce_out, 0)
                while ni < len(nxt):
                    nxt[ni]()
                    ni += 1
                if phases == "abc":
                    yd = sp.tile([128, C], F32, name="yd", tag="wstage")
                    nc.sync.dma_start(yd[:], bounce_in[0][0:128, :])
                    dump_y(yd[:])
                    continue
                emit_RS(bounce_in, bounce_out, 1)
                if phases == "all":
                    emit_E_t2(bounce_out, 1)
                tiles_cur = tiles_next

    nc.compile()
    return nc


# --------------------------------------------------------------------------
# Host-side input prep
# --------------------------------------------------------------------------

def make_core_inputs(inputs, core_id, T=T):
    b = core_id // 2
    half = core_id % 2
    h0 = NHC * half
    TH = T // 2
    f = np.asarray
    x = f(inputs["x"], dtype=np.float32)[b]
    g1 = f(inputs["ln1_g"], dtype=np.float32)
    b1_ = f(inputs["ln1_b"], dtype=np.float32)
    g2 = f(inputs["ln2_g"], dtype=np.float32)
    b2_ = f(inputs["ln2_b"], dtype=np.float32)
    Wq = f(inputs["Wq"], dtype=np.float32)
    Wk = f(inputs["Wk"], dtype=np.float32)
    Wv = f(inputs["Wv"], dtype=np.float32)
    Wp = f(inputs["Wp"], dtype=np.float32)
    bp = f(inputs["bp"], dtype=np.float32)
    W1 = f(inputs["W1"], dtype=np.float32)
    b1v = f(inputs["b1"], dtype=np.float32)
    W2 = f(inputs["W2"], dtype=np.float32)
    b2v = f(inputs["b2"], dtype=np.float32)

    def fold_w(W, h):  # [C, HS] folded with ln1 gain
        return g1[:, None] * W[h]

    def fold_b(W, h):  # ln1 bias pushed through
        return b1_ @ W[h]

    hh = [h0, h0 + 1, h0 + 2]
    wq01 = np.concatenate([fold_w(Wq, hh[0]), fold_w(Wq, hh[1])], axis=1)
    wk01 = np.concatenate([fold_w(Wk, hh[0]), fold_w(Wk, hh[1])], axis=1)
    wqk2 = np.concatenate([fold_w(Wq, hh[2]), fold_w(Wk, hh[2])], axis=1)
    wvall = np.concatenate([fold_w(Wv, hh[0]), fold_w(Wv, hh[1]),
                            fold_w(Wv, hh[2])], axis=1)
    bqcol = np.zeros((256,), np.float32)
    bqcol[0:64] = fold_b(Wq, hh[0])
    bqcol[64:128] = fold_b(Wq, hh[1])
    bqcol[128:192] = fold_b(Wq, hh[2])
    wp01 = Wp[hh[0] * HS:(hh[1] + 1) * HS, :]
    wp2 = Wp[hh[2] * HS:(hh[2] + 1) * HS, :]

    # V-bias contribution of ALL heads folds into the residual row.
    bv_all = np.concatenate([b1_ @ Wv[h] for h in range(H)])  # [C]
    bvp = bv_all @ Wp  # [C]

    w1f = g2[:, None] * W1
    b1f = b1v + b2_ @ W1

    r = np.arange(SW)[:, None]
    j = np.arange(QW)[None, :]
    maskA = (j >= r).astype(np.float32)
    maskB = (j >= r + SW).astype(np.float32)
    maskAB = np.concatenate([maskA, maskB], axis=1)

    xtok = np.concatenate([x[s:s + n] for (s, n) in core_token_slices(half)])
    return {
        "x": np.ascontiguousarray(x),
        "xpb": np.ascontiguousarray(xtok + bp + bvp),
        "wq01": np.ascontiguousarray(wq01),
        "wk01": np.ascontiguousarray(wk01),
        "wqk2": np.ascontiguousarray(wqk2),
        "wvall": np.ascontiguousarray(wvall),
        "bqcol": bqcol,
        "wp01": np.ascontiguousarray(wp01), "wp2": np.ascontiguousarray(wp2),
        "w1": np.ascontiguousarray(w1f), "b1": b1f,
        "w2": np.ascontiguousarray(W2), "b2": b2v,
        "maskAB": maskAB,
    }


_NC_CACHE = {}


def _get_nc(mm="bf16"):
    if mm not in _NC_CACHE:
        _NC_CACHE[mm] = build_block(T=T, num_devices=8, mm=mm)
    return _NC_CACHE[mm]


MM_MODE = "bf16"


def kernel(**inputs):
    nc = _get_nc(MM_MODE)
    in_maps = [make_core_inputs(inputs, c) for c in range(8)]
    res = bass_utils.run_bass_kernel_spmd(nc, in_maps, core_ids=list(range(8)))
    out = np.empty((B, T, C), dtype=np.float32)
    for c in range(8):
        b, half = c // 2, c % 2
        yc = res.results[c]["y"]
        r = 0
        for (s, n) in core_token_slices(half):
            out[b, s:s + n, :] = yc[r:r + n]
            r += n
    return out
